# revision 2
# baseline (speedup 1.0000x reference)
"""Trainium2 Bass kernel for nn_Barrier_Net (DeepSet GNN message passing).

Strategy (8 NeuronCores, SPMD):
  - Each core owns 2048 contiguous agents (16 blocks of 128 agents).
  - Host slices the sorted edge list at agent-range boundaries and pads each
    128-agent block to a uniform C chunks of 128 edges, so the compiled
    program is identical on every core (pure SPMD, no collectives).
  - phi MLP runs in transposed layout (features on partitions, edges on the
    free dim); segment-sum is a one-hot matmul (one-hot built on-device with
    an is_equal tensor_scalar against an iota tile, using host-precomputed
    block-local ids; padded edges get id -1 so they contribute nothing).
  - bp3 is folded in as a rank-1 (degree x bp3) matmul into the aggregate.
  - rho runs data-parallel over the agent dim on the same core.
  - The barrier term and br3 (negligible FLOPs) are added on the host.
"""

import numpy as np

N_AGENTS = 16384
N_EDGES = 524288
N_CORES = 8
AG_PER_CORE = N_AGENTS // N_CORES  # 2048
BLK = 128                          # agents per block
NBLK = AG_PER_CORE // BLK          # 16 blocks per core
MARGIN = 1.2 * 0.15                # barrier margin

# matmul dtype mode: "f32" (safe), "f32r" (fast fp32, N>=256), "bf16"
MM_MODE = "f32"

_compiled = {}


def _build(C):
    """Build + schedule the SPMD Bass program for C 128-edge chunks/block."""
    from contextlib import ExitStack

    import concourse.bass as bass
    import concourse.tile as tile
    from concourse import bacc, mybir

    FP = mybir.dt.float32
    E_BLK = C * 128                     # padded edges per block

    nc = bacc.Bacc("TRN2", target_bir_lowering=False, debug=False,
                   num_devices=N_CORES)

    def din(name, shape):
        return nc.dram_tensor(name, shape, FP, kind="ExternalInput").ap()

    eT = din("eT", [4, NBLK * E_BLK])
    relT = din("relT", [128, NBLK * C])
    degT = din("degT", [1, AG_PER_CORE])
    iota = din("iota", [128, 128])
    ident = din("ident", [128, 128])
    Wp1 = din("Wp1", [4, 256])
    Wp2 = din("Wp2", [256, 256])
    Wp3 = din("Wp3", [256, 64])
    Wr1 = din("Wr1", [64, 256])
    Wr2 = din("Wr2", [256, 256])
    Wr3 = din("Wr3", [256, 2])
    bp1 = din("bp1", [256, 1])
    bp2 = din("bp2", [256, 1])
    bp3 = din("bp3", [1, 64])
    br1 = din("br1", [256, 1])
    br2 = din("br2", [256, 1])
    out_d = nc.dram_tensor("out", [AG_PER_CORE, 2], FP,
                           kind="ExternalOutput").ap()

    RELU = mybir.ActivationFunctionType.Relu
    EQ = mybir.AluOpType.is_equal

    def mmcast(ap, n):
        # f32r runs 4x faster than f32 on PE when the moving free dim >= 256
        if MM_MODE == "f32r" and n >= 256:
            return ap.bitcast(mybir.dt.float32r)
        return ap

    with tile.TileContext(nc) as tc, ExitStack() as ctx:
        consts = ctx.enter_context(tc.tile_pool(name="consts", bufs=1))
        a_pool = ctx.enter_context(tc.tile_pool(name="acts", bufs=3))
        ps_mlp = ctx.enter_context(
            tc.tile_pool(name="ps_mlp", bufs=3, space="PSUM"))
        ps_sm = ctx.enter_context(
            tc.tile_pool(name="ps_sm", bufs=2, space="PSUM"))
        ps_agg = ctx.enter_context(
            tc.tile_pool(name="ps_agg", bufs=2, space="PSUM"))

        def cload(name, ap, shape=None):
            t = consts.tile(shape or list(ap.shape), FP, tag=name)
            nc.sync.dma_start(t[:], ap)
            return t

        wp1_s = cload("wp1", Wp1)
        wp2a_s = cload("wp2a", Wp2[0:128, :])
        wp2b_s = cload("wp2b", Wp2[128:256, :])
        wp3a_s = cload("wp3a", Wp3[0:128, :])
        wp3b_s = cload("wp3b", Wp3[128:256, :])
        wr1_s = cload("wr1", Wr1)
        wr2a_s = cload("wr2a", Wr2[0:128, :])
        wr2b_s = cload("wr2b", Wr2[128:256, :])
        wr3a_s = cload("wr3a", Wr3[0:128, :])
        wr3b_s = cload("wr3b", Wr3[128:256, :])
        bp1a = cload("bp1a", bp1[0:128, :])
        bp1b = cload("bp1b", bp1[128:256, :])
        bp2a = cload("bp2a", bp2[0:128, :])
        bp2b = cload("bp2b", bp2[128:256, :])
        br1a = cload("br1a", br1[0:128, :])
        br1b = cload("br1b", br1[128:256, :])
        br2a = cload("br2a", br2[0:128, :])
        br2b = cload("br2b", br2[128:256, :])
        bp3_s = cload("bp3", bp3)
        iota_s = cload("iota", iota)
        ident_s = cload("ident", ident)
        relT_s = cload("relT", relT)
        degT_s = cload("degT", degT)
        aggT_s = consts.tile([64, AG_PER_CORE], FP, tag="aggT")

        # chunk sizes per block: C//4 chunks of 512 edges + one tail
        sizes = [512] * (C // 4)
        if C % 4:
            sizes.append(128 * (C % 4))

        for j in range(NBLK):
            pagg = ps_agg.tile([128, 64], FP, tag="agg")
            first = True
            off = j * E_BLK
            sub = j * C
            for n in sizes:
                et = a_pool.tile([4, n], FP, tag="et")
                nc.sync.dma_start(et[:], eT[:, off:off + n])
                ps1a = ps_mlp.tile([128, n], FP, tag="psmlp")
                ps1b = ps_mlp.tile([128, n], FP, tag="psmlp")
                nc.tensor.matmul(ps1a[:], mmcast(wp1_s[:, 0:128], n),
                                 mmcast(et[:], n), start=True, stop=True)
                nc.tensor.matmul(ps1b[:], mmcast(wp1_s[:, 128:256], n),
                                 mmcast(et[:], n), start=True, stop=True)
                h1a = a_pool.tile([128, n], FP, tag="h1a")
                h1b = a_pool.tile([128, n], FP, tag="h1b")
                nc.scalar.activation(h1a[:], ps1a[:], RELU, bias=bp1a[:, 0:1])
                nc.scalar.activation(h1b[:], ps1b[:], RELU, bias=bp1b[:, 0:1])
                ps2a = ps_mlp.tile([128, n], FP, tag="psmlp")
                ps2b = ps_mlp.tile([128, n], FP, tag="psmlp")
                nc.tensor.matmul(ps2a[:], mmcast(wp2a_s[:, 0:128], n),
                                 mmcast(h1a[:], n), start=True, stop=False)
                nc.tensor.matmul(ps2a[:], mmcast(wp2b_s[:, 0:128], n),
                                 mmcast(h1b[:], n), start=False, stop=True)
                nc.tensor.matmul(ps2b[:], mmcast(wp2a_s[:, 128:256], n),
                                 mmcast(h1a[:], n), start=True, stop=False)
                nc.tensor.matmul(ps2b[:], mmcast(wp2b_s[:, 128:256], n),
                                 mmcast(h1b[:], n), start=False, stop=True)
                h2a = a_pool.tile([128, n], FP, tag="h2a")
                h2b = a_pool.tile([128, n], FP, tag="h2b")
                nc.scalar.activation(h2a[:], ps2a[:], RELU, bias=bp2a[:, 0:1])
                nc.scalar.activation(h2b[:], ps2b[:], RELU, bias=bp2b[:, 0:1])
                for s in range(n // 128):
                    ps3 = ps_sm.tile([128, 64], FP, tag="sm")
                    sl = slice(s * 128, (s + 1) * 128)
                    nc.tensor.matmul(ps3[:], h2a[:, sl], wp3a_s[:],
                                     start=True, stop=False)
                    nc.tensor.matmul(ps3[:], h2b[:, sl], wp3b_s[:],
                                     start=False, stop=True)
                    h3 = a_pool.tile([128, 64], FP, tag="h3")
                    nc.vector.tensor_copy(h3[:], ps3[:])
                    oh = a_pool.tile([128, 128], FP, tag="oh")
                    nc.vector.tensor_scalar(oh[:], iota_s[:],
                                            relT_s[:, sub:sub + 1], None, EQ)
                    nc.tensor.matmul(pagg[:], oh[:], h3[:],
                                     start=first, stop=False)
                    first = False
                    sub += 1
                off += n
            # fold in bp3: agg += deg (x) bp3   (rank-1)
            nc.tensor.matmul(pagg[:], degT_s[:, j * 128:(j + 1) * 128],
                             bp3_s[:], start=first, stop=True)
            agg_sb = a_pool.tile([128, 64], FP, tag="aggsb")
            nc.vector.tensor_copy(agg_sb[:], pagg[:])
            pst = ps_sm.tile([64, 128], FP, tag="sm")
            nc.tensor.transpose(pst[:], agg_sb[:], ident_s[:])
            nc.vector.tensor_copy(aggT_s[:, j * 128:(j + 1) * 128], pst[:])

        # rho: data-parallel over agents, 512 at a time
        for g in range(AG_PER_CORE // 512):
            sl = slice(g * 512, (g + 1) * 512)
            pr1a = ps_mlp.tile([128, 512], FP, tag="psmlp")
            pr1b = ps_mlp.tile([128, 512], FP, tag="psmlp")
            nc.tensor.matmul(pr1a[:], mmcast(wr1_s[:, 0:128], 512),
                             mmcast(aggT_s[:, sl], 512), start=True, stop=True)
            nc.tensor.matmul(pr1b[:], mmcast(wr1_s[:, 128:256], 512),
                             mmcast(aggT_s[:, sl], 512), start=True, stop=True)
            r1a = a_pool.tile([128, 512], FP, tag="h1a")
            r1b = a_pool.tile([128, 512], FP, tag="h1b")
            nc.scalar.activation(r1a[:], pr1a[:], RELU, bias=br1a[:, 0:1])
            nc.scalar.activation(r1b[:], pr1b[:], RELU, bias=br1b[:, 0:1])
            pr2a = ps_mlp.tile([128, 512], FP, tag="psmlp")
            pr2b = ps_mlp.tile([128, 512], FP, tag="psmlp")
            nc.tensor.matmul(pr2a[:], mmcast(wr2a_s[:, 0:128], 512),
                             mmcast(r1a[:], 512), start=True, stop=False)
            nc.tensor.matmul(pr2a[:], mmcast(wr2b_s[:, 0:128], 512),
                             mmcast(r1b[:], 512), start=False, stop=True)
            nc.tensor.matmul(pr2b[:], mmcast(wr2a_s[:, 128:256], 512),
                             mmcast(r1a[:], 512), start=True, stop=False)
            nc.tensor.matmul(pr2b[:], mmcast(wr2b_s[:, 128:256], 512),
                             mmcast(r1b[:], 512), start=False, stop=True)
            r2a = a_pool.tile([128, 512], FP, tag="h2a")
            r2b = a_pool.tile([128, 512], FP, tag="h2b")
            nc.scalar.activation(r2a[:], pr2a[:], RELU, bias=br2a[:, 0:1])
            nc.scalar.activation(r2b[:], pr2b[:], RELU, bias=br2b[:, 0:1])
            for s in range(4):
                pso = ps_sm.tile([128, 2], FP, tag="sm")
                ssl = slice(s * 128, (s + 1) * 128)
                nc.tensor.matmul(pso[:], r2a[:, ssl], wr3a_s[:],
                                 start=True, stop=False)
                nc.tensor.matmul(pso[:], r2b[:, ssl], wr3b_s[:],
                                 start=False, stop=True)
                o_sb = a_pool.tile([128, 2], FP, tag="osb")
                nc.vector.tensor_copy(o_sb[:], pso[:])
                nc.sync.dma_start(out_d[g * 512 + s * 128:
                                        g * 512 + (s + 1) * 128, :], o_sb[:])

    nc.compile()
    return nc


def _prep_inputs(edge_feats, segment_ids, ws):
    """Host-side shard + pad. Returns (C, in_maps)."""
    seg = np.asarray(segment_ids).astype(np.int64)
    ef = np.asarray(edge_feats, dtype=np.float32)
    bounds = np.searchsorted(seg, np.arange(0, N_AGENTS + 1, BLK))
    counts = np.diff(bounds)                      # edges per 128-agent block
    C = int(np.ceil(counts.max() / 128))
    E_BLK = C * 128

    iota = np.tile(np.arange(128, dtype=np.float32), (128, 1))
    ident = np.eye(128, dtype=np.float32)
    const_w = {
        "iota": iota, "ident": ident,
        "Wp1": ws["Wp1"], "Wp2": ws["Wp2"], "Wp3": ws["Wp3"],
        "Wr1": ws["Wr1"], "Wr2": ws["Wr2"], "Wr3": ws["Wr3"],
        "bp1": ws["bp1"].reshape(256, 1), "bp2": ws["bp2"].reshape(256, 1),
        "bp3": ws["bp3"].reshape(1, 64),
        "br1": ws["br1"].reshape(256, 1), "br2": ws["br2"].reshape(256, 1),
    }
    const_w = {k: np.ascontiguousarray(v, dtype=np.float32)
               for k, v in const_w.items()}

    in_maps = []
    for i in range(N_CORES):
        eT = np.zeros((4, NBLK * E_BLK), np.float32)
        relT = np.full((128, NBLK * C), -1.0, np.float32)
        deg = np.zeros(AG_PER_CORE, np.float32)
        for j in range(NBLK):
            g = NBLK * i + j
            s, e = bounds[g], bounds[g + 1]
            cnt = e - s
            eT[:, j * E_BLK: j * E_BLK + cnt] = ef[s:e].T
            rel = np.full(E_BLK, -1.0, np.float32)
            rel[:cnt] = (seg[s:e] - 128 * g).astype(np.float32)
            relT[:, j * C:(j + 1) * C] = rel.reshape(C, 128).T
            np.add.at(deg, seg[s:e] - AG_PER_CORE * i, 1.0)
        m = {"eT": eT, "relT": relT, "degT": deg.reshape(1, -1)}
        m.update(const_w)
        in_maps.append(m)
    return C, in_maps


def _host_barrier(edge_feats, segment_ids):
    ef = np.asarray(edge_feats, dtype=np.float64)
    seg = np.asarray(segment_ids).astype(np.int64)
    p = ef[:, :2]
    d = np.sqrt((p * p).sum(1, keepdims=True))
    contrib = -(p / d) / (d - MARGIN)
    barrier = np.zeros((N_AGENTS, 2), np.float64)
    np.add.at(barrier, seg, contrib)
    return barrier


def kernel(edge_feats, segment_ids, Wp1, bp1, Wp2, bp2, Wp3, bp3,
           Wr1, br1, Wr2, br2, Wr3, br3, _trace=False):
    from concourse.bass_utils import run_bass_kernel_spmd

    ws = dict(Wp1=Wp1, bp1=bp1, Wp2=Wp2, bp2=bp2, Wp3=Wp3, bp3=bp3,
              Wr1=Wr1, br1=br1, Wr2=Wr2, br2=br2, Wr3=Wr3, br3=br3)
    ws = {k: np.asarray(v, dtype=np.float32) for k, v in ws.items()}
    C, in_maps = _prep_inputs(edge_feats, segment_ids, ws)
    if C not in _compiled:
        _compiled[C] = _build(C)
    nc = _compiled[C]
    res = run_bass_kernel_spmd(nc, in_maps, list(range(N_CORES)),
                               trace=_trace)
    out = np.concatenate([res.results[i]["out"] for i in range(N_CORES)], 0)
    out = (out.astype(np.float64) + _host_barrier(edge_feats, segment_ids)
           + np.asarray(ws["br3"], np.float64).reshape(1, 2))
    if _trace:
        kernel._last_results = res
    return out.astype(np.float32)
